# revision 76
# baseline (speedup 1.0000x reference)
"""Trainium2 Bass kernel for a transformer block: MLA attention + top-2 MoE (8 experts).

Sharding (8 NeuronCores):
  Launch 1 (head-parallel attention): core c = (batch b=c//4, head-group
    g=c%4 of 4 heads). The host applies LN1 and ships h^T pre-transposed in
    fp8e4 (x4); Q/CKV projections and the out-projection run as fp8 DoubleRow
    matmuls (2 k-subtiles per instruction), the KV up-projection stays bf16
    for accuracy and is computed in both layouts on the PE (no DRAM
    roundtrip). Causal softmax uses a transposed-scores layout: exp() on the
    Act engine writes fp8 probabilities, multiplicative 0/1 masks zero the
    diagonal tiles, and P@V accumulates numerator plus an augmented
    ones-column denominator in PSUM via fp8 DoubleRow over k-tile pairs.
    Emission is software-pipelined (scores run DPIPE stages ahead of P@V)
    with next-chunk projection work injected into the PE stream as filler.
    Partial out-projections attn_g @ Wo[g-rows] go back per core; host sums.
  Host: xnew = x + sum(partials); LN2; gate logits; top-2 softmax; per-expert
    token gather (the "all-to-all dispatch").
  Launch 2 (expert-parallel MLP): core e = expert e on its gathered tokens,
    fp8e4 DoubleRow GEMMs with power-of-2 scale folding (x*4 @ W1*64, gelu
    via Act with 1/256 descale, fp8 hidden @ W2*32, 1/32 descale on output).
    Host applies the top-2 combine weights and b2 during scatter-add.
"""

import numpy as np
import ml_dtypes

import concourse.bass as bass
import concourse.bacc as bacc
import concourse.mybir as mybir
from concourse.tile import TileContext
from concourse.masks import make_identity
from concourse.bass_utils import run_bass_kernel_spmd

F32 = mybir.dt.float32
BF16 = mybir.dt.bfloat16
AF = mybir.ActivationFunctionType

B, S, D = 2, 2048, 1024
H, DH, DL = 16, 64, 512
E, DFF, TOPK = 8, 2048, 2
HC = 4            # heads per core
HDC = HC * DH     # 256
EPS = 1e-5
NEG = -1.0e30

_cache = {}


# l1 fp8 scales: h ×SXA, Wq/Wdkv ×SWP (q/ckv psum = SXA*SWP * true)
SXA, SWP = 4.0, 64.0
SAT, SWO = 4.0, 64.0   # attn out x4 into fp8; Wo x64 into fp8
GSZ = 4     # k-tiles per softmax group (psS bank budget)
DPIPE = 2   # scores-ahead-of-PV software pipeline depth

F8 = mybir.dt.float8e4
DR = mybir.MatmulPerfMode.DoubleRow

# fp8 scale factors (powers of 2, folded back out via activation scale)
SX, SW1, SH, SW2 = 4.0, 64.0, 16.0, 32.0



def build_l1():
    """Head-parallel MLA attention. Host pre-computes LN1 and passes hT fp8.

    Per core: q/ckv projections in fp8 DoubleRow; kv up-projection in bf16
    (both layouts computed on PE, no DRAM roundtrip); causal softmax with
    multiplicative 0/1 masks applied to exp(scores); denominators via an
    augmented ones column; out-projection partials DMAed straight from PSUM.
    """
    nc = bacc.Bacc()
    hT8 = nc.dram_tensor("hT8", [D, S], F8, kind="ExternalInput")
    wq = nc.dram_tensor("wq", [D, HDC], F8, kind="ExternalInput")
    wdkv = nc.dram_tensor("wdkv", [D, DL], F8, kind="ExternalInput")
    wukv = nc.dram_tensor("wukv", [DL, HDC], BF16, kind="ExternalInput")
    wo = nc.dram_tensor("wo", [HDC, D], F8, kind="ExternalInput")
    mmask = nc.dram_tensor("mmask", [2 * 128, 256], F8, kind="ExternalInput")
    xpart = nc.dram_tensor("xpart", [S, D], BF16, kind="ExternalOutput")

    PRJ = 1.0 / (SXA * SWP)

    with TileContext(nc) as tc:
        import contextlib
        with contextlib.ExitStack() as ctx:
            singles = ctx.enter_context(tc.tile_pool(name="singles", bufs=1))
            wpool = ctx.enter_context(tc.tile_pool(name="wpool", bufs=1))
            big = ctx.enter_context(tc.tile_pool(name="big", bufs=1))
            work = ctx.enter_context(tc.tile_pool(name="work", bufs=16))
            psA = ctx.enter_context(tc.tile_pool(name="psA", bufs=2, space="PSUM"))
            psS = ctx.enter_context(tc.tile_pool(name="psS", bufs=2, space="PSUM"))
            psO = ctx.enter_context(tc.tile_pool(name="psO", bufs=2, space="PSUM"))

            # loads, critical-path first
            hTs = [big.tile([128, 8, 512], F8, name=f"hT{rc}", tag=f"hT{rc}")
                   for rc in range(4)]

            def _load_h(rc):
                nc.sync.dma_start(
                    out=hTs[rc],
                    in_=hT8[:, rc * 512:(rc + 1) * 512].rearrange(
                        "(j s p) n -> p (j s) n", j=4, s=2))

            _load_h(0)
            wdkv_sb = wpool.tile([128, 8, DL], F8, name="wdkv", tag="wdkv")
            nc.sync.dma_start(
                out=wdkv_sb, in_=wdkv[:, :].rearrange("(j s p) n -> p (j s) n",
                                                      j=4, s=2))
            wukv_sb = [wpool.tile([128, HDC], BF16, name=f"wukv{i}",
                                  tag=f"wukv{i}") for i in range(4)]
            for i in range(4):
                nc.sync.dma_start(out=wukv_sb[i],
                                  in_=wukv[i * 128:(i + 1) * 128, :])
            wq_sb = wpool.tile([128, 8, HDC], F8, name="wq", tag="wq")
            nc.sync.dma_start(
                out=wq_sb, in_=wq[:, :].rearrange("(j s p) n -> p (j s) n",
                                                  j=4, s=2))
            masks = singles.tile([128, 2, 256], F8, name="masks", tag="masks")
            nc.sync.dma_start(out=masks,
                              in_=mmask[:, :].rearrange("(v p) n -> p v n", v=2))
            ident = singles.tile([128, 128], BF16, name="ident", tag="ident")
            make_identity(nc, ident)
            wo_sb = wpool.tile([128, 2, D], F8, name="wo", tag="wo")

            qT = [big.tile([128, S], BF16, name=f"qT{i}", tag=f"qT{i}")
                  for i in range(2)]
            ckvT = [big.tile([128, S], BF16, name=f"ckvT{i}", tag=f"ckvT{i}")
                    for i in range(4)]
            kvT = [big.tile([128, S], BF16, name=f"kvT{i}", tag=f"kvT{i}")
                   for i in range(2)]
            kva2 = [big.tile([128, 2, HC, DH + 1], F8, name=f"kva{t}",
                             tag=f"kva{t}") for t in range(8)]
            attn_sb = [big.tile([128, HDC], BF16, name=f"attn{i}", tag=f"attn{i}")
                       for i in range(16)]
            attnT2 = big.tile([128, 2, S], F8, name="attnT2", tag="attnT2")

            def _u_ckv(rc, dlt):
                cs = slice(rc * 512, (rc + 1) * 512)
                ps = psA.tile([128, 512], F32, name="psB", tag="psB")
                for j in range(4):
                    nc.tensor.matmul(ps,
                                     wdkv_sb[:, 2 * j:2 * j + 2,
                                             dlt * 128:(dlt + 1) * 128],
                                     hTs[rc][:, 2 * j:2 * j + 2, :],
                                     start=(j == 0), stop=(j == 3),
                                     perf_mode=DR)
                if rc == 0:  # Act is idle before the first exp
                    nc.scalar.activation(out=ckvT[dlt][:, cs], in_=ps,
                                         func=AF.Copy, scale=PRJ)
                else:
                    nc.vector.tensor_scalar(out=ckvT[dlt][:, cs], in0=ps,
                                            scalar1=PRJ, scalar2=None,
                                            op0=mybir.AluOpType.mult)

            def _u_q(rc, ht):
                cs = slice(rc * 512, (rc + 1) * 512)
                ps = psA.tile([128, 512], F32, name="psB", tag="psB")
                for j in range(4):
                    nc.tensor.matmul(ps,
                                     wq_sb[:, 2 * j:2 * j + 2,
                                           ht * 128:(ht + 1) * 128],
                                     hTs[rc][:, 2 * j:2 * j + 2, :],
                                     start=(j == 0), stop=(j == 3),
                                     perf_mode=DR)
                if rc == 0:
                    nc.scalar.activation(out=qT[ht][:, cs], in_=ps,
                                         func=AF.Copy, scale=PRJ)
                else:
                    nc.vector.tensor_scalar(out=qT[ht][:, cs], in0=ps,
                                            scalar1=PRJ, scalar2=None,
                                            op0=mybir.AluOpType.mult)

            def _u_kvT(rc, ht):
                cs = slice(rc * 512, (rc + 1) * 512)
                ps = psA.tile([128, 512], F32, name="psB", tag="psB")
                for dlt in range(4):
                    nc.tensor.matmul(ps,
                                     wukv_sb[dlt][:, ht * 128:(ht + 1) * 128],
                                     ckvT[dlt][:, cs],
                                     start=(dlt == 0), stop=(dlt == 3))
                if rc == 0:
                    nc.scalar.activation(out=kvT[ht][:, cs], in_=ps,
                                         func=AF.Copy)
                else:
                    nc.vector.tensor_copy(out=kvT[ht][:, cs], in_=ps)

            def _u_kva(rc, kt):
                kp, ks = kt // 2, kt % 2
                nc.gpsimd.memset(kva2[kp][:, ks, :, DH:DH + 1], 1.0)
                ps = psA.tile([128, 512], F32, name="psB", tag="psB")
                for dlt in range(4):
                    nc.tensor.matmul(ps[:, 0:HDC],
                                     ckvT[dlt][:, kt * 128:(kt + 1) * 128],
                                     wukv_sb[dlt],
                                     start=(dlt == 0), stop=(dlt == 3))
                nc.vector.tensor_copy(
                    out=kva2[kp][:, ks, :, 0:DH],
                    in_=ps[:, 0:HDC].rearrange("p (h d) -> p h d", h=HC))

            def proj_units(rc):
                from functools import partial
                u = []
                for dlt in range(4):
                    u.append(partial(_u_ckv, rc, dlt))
                for ht in range(2):
                    u.append(partial(_u_q, rc, ht))
                for ht in range(2):
                    u.append(partial(_u_kvT, rc, ht))
                for kt in range(4 * rc, 4 * rc + 4):
                    u.append(partial(_u_kva, rc, kt))
                return u

            def scores(qp, h, g0):
                nkt = 2 * qp + 2
                gn = min(GSZ, nkt - g0)
                tI, pO = h // 2, (h % 2) * 64
                ps = psS.tile([128, 256 * GSZ], F32, name="psS", tag="psS")
                for kl in range(gn):
                    kt = g0 + kl
                    nc.tensor.matmul(
                        ps[:, kl * 256:(kl + 1) * 256],
                        kvT[tI][pO:pO + 64, kt * 128:(kt + 1) * 128],
                        qT[tI][pO:pO + 64, qp * 256:(qp + 1) * 256],
                        start=True, stop=True)
                pbT = work.tile([128, 256 * GSZ], F8, name="pbT", tag="pbT")
                nc.scalar.activation(out=pbT[:, 0:gn * 256],
                                     in_=ps[:, 0:gn * 256], func=AF.Exp,
                                     scale=1.0 / (DH ** 0.5))
                for kl in range(gn):
                    kt = g0 + kl
                    if kt >= 2 * qp:  # diagonal: zero masked probs
                        nc.gpsimd.tensor_mul(
                            out=pbT[:, kl * 256:(kl + 1) * 256],
                            in0=pbT[:, kl * 256:(kl + 1) * 256],
                            in1=masks[:, kt - 2 * qp, :])
                return pbT

            Ps = {}

            def pv(qp, h, g0, pbT):
                nkt = 2 * qp + 2
                gn = min(GSZ, nkt - g0)
                P0, P1 = Ps[(qp, h)]
                vk = pbT.rearrange("p (k j c) -> p k j c", k=GSZ, j=2)
                for kl in range(0, gn, 2):
                    kt = g0 + kl
                    for j, P in ((0, P0), (1, P1)):
                        nc.tensor.matmul(
                            P,
                            vk[:, kl:kl + 2, j, :],
                            kva2[kt // 2][:, :, h, :],
                            start=(kt == 0), stop=(kt + 2 == nkt),
                            perf_mode=DR)
                if g0 + GSZ >= nkt:  # head finished: normalize + store
                    del Ps[(qp, h)]
                    rec = work.tile([128, 2], F32, name="rec", tag="rec")
                    nc.vector.reciprocal(out=rec[:, 0:1], in_=P0[:, DH:DH + 1])
                    nc.vector.reciprocal(out=rec[:, 1:2], in_=P1[:, DH:DH + 1])
                    for j, P in ((0, P0), (1, P1)):
                        nc.vector.tensor_scalar_mul(
                            out=attn_sb[2 * qp + j][:, h * DH:(h + 1) * DH],
                            in0=P[:, 0:DH], scalar1=rec[:, j:j + 1])
                    if h == HC - 1:
                        post(qp)

            def post(qp):
                for qt in (2 * qp, 2 * qp + 1):
                    for hd in range(2):
                        pt = psA.tile([128, 128], BF16, name="ptT", tag="psB")
                        nc.tensor.transpose(
                            pt, attn_sb[qt][:, hd * 128:(hd + 1) * 128], ident)
                        nc.vector.tensor_scalar(
                            out=attnT2[:, hd, qt * 128:(qt + 1) * 128], in0=pt,
                            scalar1=SAT, scalar2=None,
                            op0=mybir.AluOpType.mult)
                    xp = work.tile([128, D], BF16, name="xp", tag="xp")
                    for dh2 in range(2):
                        ps = psA.tile([128, 512], F32, name="psB", tag="psB")
                        nc.tensor.matmul(
                            ps, attnT2[:, :, qt * 128:(qt + 1) * 128],
                            wo_sb[:, :, dh2 * 512:(dh2 + 1) * 512],
                            start=True, stop=True, perf_mode=DR)
                        nc.vector.tensor_scalar(
                            out=xp[:, dh2 * 512:(dh2 + 1) * 512], in0=ps,
                            scalar1=1.0 / (SAT * SWO), scalar2=None,
                            op0=mybir.AluOpType.mult)
                        nc.sync.dma_start(
                            out=xpart[qt * 128:(qt + 1) * 128,
                                      dh2 * 512:(dh2 + 1) * 512],
                            in_=xp[:, dh2 * 512:(dh2 + 1) * 512])

            for u in proj_units(0):
                u()
            for rc in range(1, 4):
                _load_h(rc)
            nc.sync.dma_start(
                out=wo_sb,
                in_=wo[:, :].rearrange("(s p) n -> p s n", s=2))

            # one global software-pipelined stage stream across all qp
            pend = []
            for rc in range(4):
                nxt = iter(proj_units(rc + 1) if rc < 3 else [])

                def inject(k=1):
                    for _ in range(k):
                        u = next(nxt, None)
                        if u is not None:
                            u()

                for qp in (2 * rc, 2 * rc + 1):
                    nkt = 2 * qp + 2
                    for h in range(HC):
                        for g0 in range(0, nkt, GSZ):
                            if g0 == 0:
                                Ps[(qp, h)] = (
                                    psO.tile([128, DH + 1], F32, name="P0",
                                             tag="Pacc"),
                                    psO.tile([128, DH + 1], F32, name="P1",
                                             tag="Pacc"))
                            pbT = scores(qp, h, g0)
                            pend.append((qp, h, g0, pbT))
                            if len(pend) > DPIPE:
                                pv(*pend.pop(0))
                            inject(2 if qp == 2 * rc else 1)
                    for st in pend:
                        pv(*st)
                    pend = []
                for u in nxt:
                    u()
    nc.compile()
    return nc
def build_l2(capT: int):
    """Expert MLP on gathered tokens, fp8e4 DoubleRow matmuls.

    yT = gelu(Xe@W1 + b1) @ W2 / SW2  (un-combined); host applies the top-k
    combine weight and adds w*b2 during scatter. Xe is pre-scaled by SX,
    W1 by SW1, W2 by SW2 on the host.
    """
    nc = bacc.Bacc()
    xeT = nc.dram_tensor("xeT", [D, capT], F8, kind="ExternalInput")
    w1 = nc.dram_tensor("w1", [D, DFF], F8, kind="ExternalInput")
    w2 = nc.dram_tensor("w2", [DFF, D], F8, kind="ExternalInput")
    b1 = nc.dram_tensor("b1", [128, DFF // 128], F32, kind="ExternalInput")
    yT = nc.dram_tensor("yT", [D, capT], BF16, kind="ExternalOutput")

    chunks = [(0, min(256, capT))]
    off = chunks[0][1]
    while off < capT:
        n = min(512, capT - off)
        chunks.append((off, n))
        off += n

    with TileContext(nc) as tc:
        import contextlib
        with contextlib.ExitStack() as ctx:
            singles = ctx.enter_context(tc.tile_pool(name="singles", bufs=1))
            wpool = ctx.enter_context(tc.tile_pool(name="wpool", bufs=1))
            big = ctx.enter_context(tc.tile_pool(name="big", bufs=1))
            outp = ctx.enter_context(tc.tile_pool(name="outp", bufs=8))
            psp = ctx.enter_context(tc.tile_pool(name="psp", bufs=8, space="PSUM"))
            psq = psp

            # single-DMA loads: xe per chunk, w1 in two ft-halves, w2 whole,
            # issued in critical-path-first order
            xec = [big.tile([128, 8, n], F8, name=f"xe{c}", tag=f"xe{c}")
                   for c, (off, n) in enumerate(chunks)]
            w1s = [wpool.tile([128, 8, DFF // 4], F8, name=f"w1s{h}",
                              tag=f"w1s{h}") for h in range(4)]
            w2s = wpool.tile([128, 16, D], F8, name="w2s", tag="w2s")
            b1s = singles.tile([128, DFF // 128], F32, name="b1s", tag="b1s")

            def _load_xe(c):
                off, n = chunks[c]
                nc.sync.dma_start(
                    out=xec[c],
                    in_=xeT[:, off:off + n].rearrange(
                        "(j s p) n -> p (j s) n", j=4, s=2))

            _load_xe(0)
            q = DFF // 4
            nc.sync.dma_start(
                out=w1s[0],
                in_=w1[:, 0:q].rearrange("(j s p) n -> p (j s) n", j=4, s=2))
            nc.sync.dma_start(out=b1s, in_=b1[:, :])
            for h in range(1, 4):
                nc.sync.dma_start(
                    out=w1s[h],
                    in_=w1[:, h * q:(h + 1) * q].rearrange(
                        "(j s p) n -> p (j s) n", j=4, s=2))
            for c in range(1, len(chunks)):
                _load_xe(c)
            nc.sync.dma_start(
                out=w2s,
                in_=w2[:, :].rearrange("(j s p) n -> p (j s) n", j=8, s=2))

            hid2 = [[big.tile([128, 2, n], F8, name=f"hid2_{c}_{j}",
                              tag=f"hid2_{c}_{j}") for j in range(8)]
                    for c, (off, n) in enumerate(chunks)]
            for c, (off, n) in enumerate(chunks):
                for ft in range(16):
                    h, fl = ft // 4, ft % 4
                    ps = psp.tile([128, 512], F32, name="ps1", tag="ps")
                    for j in range(4):
                        nc.tensor.matmul(ps[:, 0:n],
                                         w1s[h][:, 2 * j:2 * j + 2,
                                                fl * 128:(fl + 1) * 128],
                                         xec[c][:, 2 * j:2 * j + 2, 0:n],
                                         start=(j == 0), stop=(j == 3),
                                         perf_mode=DR)
                    nc.scalar.activation(out=hid2[c][ft // 2][:, ft % 2, 0:n],
                                         in_=ps[:, 0:n], func=AF.Gelu,
                                         bias=b1s[:, ft:ft + 1],
                                         scale=1.0 / (SX * SW1))
            for c, (off, n) in enumerate(chunks):
                for dt in range(8):
                    ps = psq.tile([128, 512], F32, name="ps2", tag="ps")
                    for j in range(8):
                        nc.tensor.matmul(ps[:, 0:n],
                                         w2s[:, 2 * j:2 * j + 2,
                                             dt * 128:(dt + 1) * 128],
                                         hid2[c][j][:, :, 0:n],
                                         start=(j == 0), stop=(j == 7),
                                         perf_mode=DR)
                    ot = outp.tile([128, 512], BF16, name="ot", tag="ot")
                    if dt % 2 == 0:
                        nc.vector.tensor_scalar(out=ot[:, 0:n], in0=ps[:, 0:n],
                                                scalar1=1.0 / SW2, scalar2=None,
                                                op0=mybir.AluOpType.mult)
                    else:
                        nc.scalar.activation(out=ot[:, 0:n], in_=ps[:, 0:n],
                                             func=AF.Copy, scale=1.0 / SW2)
                    nc.sync.dma_start(out=yT[dt * 128:(dt + 1) * 128, off:off + n],
                                      in_=ot[:, 0:n])
    nc.compile()
    return nc


def _bf(a):
    return np.ascontiguousarray(np.asarray(a).astype(ml_dtypes.bfloat16))


def _f8(a, scale):
    a = np.asarray(a, np.float32) * scale
    np.clip(a, -240.0, 240.0, out=a)
    return np.ascontiguousarray(a.astype(ml_dtypes.float8_e4m3))


def _f32c(a):
    return np.ascontiguousarray(np.asarray(a, np.float32))


def kernel(x, mask, ln1_scale, ln1_bias, Wq, Wdkv, Wukv, Wo,
           ln2_scale, ln2_bias, Wgate, bgate, We1, be1, We2, be2,
           _collect=None):
    x = np.asarray(x, np.float32)
    g1v = np.asarray(ln1_scale, np.float32)
    b1v = np.asarray(ln1_bias, np.float32)

    # LN1 on host (elementwise prep); kernel gets hT pre-transposed in fp8
    mu = x.mean(axis=2, keepdims=True)
    var = ((x - mu) ** 2).mean(axis=2, keepdims=True)
    h1 = ((x - mu) / np.sqrt(var + EPS)) * g1v + b1v
    hT8 = [_f8(h1[b].T, SXA) for b in range(B)]

    # multiplicative 0/1 causal masks for the two diagonal 128k x 256q tiles
    ii = np.arange(128)[:, None]
    jj = np.arange(256)[None, :]
    m0 = (jj >= ii).astype(np.float32)
    m1 = (jj >= ii + 128).astype(np.float32)
    mmask = np.ascontiguousarray(
        np.concatenate([m0, m1], axis=0).astype(ml_dtypes.float8_e4m3))

    wdkv_8 = _f8(np.asarray(Wdkv, np.float32), SWP)
    l1_maps = []
    for c in range(8):
        b, g = c // 4, c % 4
        cs = slice(g * HDC, (g + 1) * HDC)
        l1_maps.append({
            "hT8": hT8[b],
            "wq": _f8(np.asarray(Wq, np.float32)[:, cs], SWP),
            "wdkv": wdkv_8,
            "wukv": _bf(np.asarray(Wukv)[:, cs]),
            "wo": _f8(np.asarray(Wo, np.float32)[cs, :], SWO),
            "mmask": mmask,
        })

    if "l1" not in _cache:
        _cache["l1"] = build_l1()
    r1 = run_bass_kernel_spmd(_cache["l1"], l1_maps, core_ids=list(range(8)))
    if _collect is not None:
        _collect["r1"] = r1

    xnew = x.copy().reshape(B, S, D)
    for c in range(8):
        xnew[c // 4] += r1.results[c]["xpart"].astype(np.float32)
    xf = xnew.reshape(B * S, D)

    # LN2 + gate on host (fp32)
    mu = xf.mean(axis=1, keepdims=True)
    var = ((xf - mu) ** 2).mean(axis=1, keepdims=True)
    h2 = ((xf - mu) / np.sqrt(var + EPS) * np.asarray(ln2_scale, np.float32)
          + np.asarray(ln2_bias, np.float32)).astype(np.float32)
    logits = h2 @ np.asarray(Wgate, np.float32) + np.asarray(bgate, np.float32)
    order = np.argsort(-logits, axis=1, kind="stable")[:, :TOPK]
    tv = np.take_along_axis(logits, order, axis=1)
    ex = np.exp(tv - tv.max(axis=1, keepdims=True))
    wtop = (ex / ex.sum(axis=1, keepdims=True)).astype(np.float32)

    idxs, wts = [], []
    for e in range(E):
        m_e = (order == e)
        rows = np.nonzero(m_e.any(axis=1))[0]
        w_e = (wtop * m_e).sum(axis=1)[rows]
        idxs.append(rows)
        wts.append(w_e.astype(np.float32))
    maxc = max(len(r) for r in idxs)
    capT = max(512, ((maxc + 127) // 128) * 128)

    w1_b, w2_b = np.asarray(We1), np.asarray(We2)
    be1_f, be2_f = np.asarray(be1, np.float32), np.asarray(be2, np.float32)
    l2_maps = []
    for e in range(E):
        n = len(idxs[e])
        xeT = np.zeros((D, capT), ml_dtypes.float8_e4m3)
        xeT[:, :n] = _f8(h2[idxs[e]].T, SX)
        l2_maps.append({
            "xeT": np.ascontiguousarray(xeT),
            "w1": _f8(w1_b[e], SW1),
            "w2": _f8(w2_b[e], SW2),
            "b1": np.ascontiguousarray(be1_f[e].reshape(DFF // 128, 128).T),
        })

    key = ("l2", capT)
    if key not in _cache:
        _cache[key] = build_l2(capT)
    r2 = run_bass_kernel_spmd(_cache[key], l2_maps, core_ids=list(range(8)))
    if _collect is not None:
        _collect["r2"] = r2

    out = xf.copy()
    for e in range(E):
        n = len(idxs[e])
        out[idxs[e]] += wts[e][:, None] * (
            r2.results[e]["yT"][:, :n].T.astype(np.float32)
            + be2_f[e][None, :])
    return out.reshape(B, S, D).astype(np.float32)

